# revision 39
# baseline (speedup 1.0000x reference)
"""DeepAR (2-layer LSTM, B=1024, W=288, H=128) forward on 8 Trainium2 cores.

Pure data-parallel: batch 1024 -> 128 per core; weights replicated.

Device layout is "transposed activations": every on-chip tensor is
(feature_dim = partitions, batch = free).  States C = 2c, H = 2h
(weights consuming h are pre-halved; i/f/o gate rows pre-halved), and
sigma(x) = (tanh(x/2)+1)/2 so every gate nonlinearity is a tanh.

The teacher phase is bound by the L1 recurrence's serial latency
(mm tail -> nonlinearity -> c update -> h -> next mm), so the L1 cell
runs its gate tanh as a degree-7 odd polynomial DIRECTLY ON THE VECTOR
ENGINE (custom DVE op reading PSUM), skipping the scalar engine's
~800ns round trip; only To = tanh(o) runs on ACT, concurrently and off
the critical path.  The h op is a custom DVE op computing
H = C*poly5(C^2)*(To+1), fusing the output-gate sigmoid.

L2's teacher cell lags L1 by OFF steps but still must sustain a
1-step cadence through its own recurrence, so it keeps the gate tanh
on ACT (two instructions, freeing DVE stream time for the L1 chain)
with the same DVE uv/c/h tail.  Scheduling floors (tile_wait_until's
logical clock) keep L2's DVE ops out of the middle of L1's
dependency chain, and floor the otherwise dependency-free filler
matmuls to their intended step.

The prediction phase is two serial cells per step; both use the fast
cell.  Filler matmuls keep the PE's activity window hot so the
dependent wfb/wi1 matmul tails run at the full 2.4 GHz p-state
(measured 56ns vs 107ns per 128-row matmul).

Prediction-phase feedback (prev_y = mean_{t-1}) is folded into the
recurrence as a rank-1 matrix Wfb = Wi0[:,0] (x) (0.5*meanW) applied to
H2; means are computed on the host from the exported H2 states.
"""

import ml_dtypes
import numpy as np

BF16 = ml_dtypes.bfloat16

B = 1024
SEQ, PRED = 192, 96
W = SEQ + PRED  # 288
HID = 128
NCORES = 8
BS = B // NCORES  # 128
IN = 67
KX = IN + 2  # + ones row (bias1) + indicator row (pred feedback bias)
G4 = 4 * HID  # 512
# torch gate order (i, f, g, o) -> device order (g, f, i, o)
GATE_PERM = [2, 1, 0, 3]
X_CHUNK = 16  # scan steps per input-DMA chunk
WOFF = {"wi0": 0, "wh0": 512, "wi1": 1024, "wh1": 1536, "wfb": 2048,
        "b2m": 2560, "bones": 2688}
WCOLS = 2688 + 512  # 3200

# degree-7 odd minimax of tanh(x) on |x|<=1.45 (gate preacts stay under 1.3)
G0, G1, G2, G3 = 0.99743805, -0.31383234, 0.09074439, -0.01307289
# degree-5 odd minimax of tanh(X/2) ~= X*(H0 + H1*X^2 + H2*X^4), |X|<=1.85
# (h op emits H = 2h = (To+1) * tanh(C/2), C=2c stays under ~1.75)
H0, H1, H2 = 0.99825091 / 2, -0.07862302 / 2, 0.00500062 / 2


def _perm_rows(w):
    """(4H, X) or (4H,) -> gate-permuted; f/i/o rows halved (tanh trick)."""
    w = w.reshape(4, HID, -1) if w.ndim == 2 else w.reshape(4, HID, 1)
    w = w[GATE_PERM].astype(np.float64).copy()  # (g, f, i, o)
    w[1] *= 0.5  # f
    w[2] *= 0.5  # i
    w[3] *= 0.5  # o
    return w  # (4, HID, X)


def _as_blocksT(w4):
    """(4, HID, K) -> (K, 4*HID) with gate blocks along columns (lhsT form)."""
    k = w4.shape[2]
    out = np.zeros((k, G4), np.float64)
    for g in range(4):
        out[:, g * HID:(g + 1) * HID] = w4[g].T
    return out


def host_prep(inputs):
    """All data-movement-only preprocessing + weight folding. Returns dict."""
    f32 = np.float32
    ge = np.asarray(inputs["given_enc"], f32)
    x_enc = np.asarray(inputs["x_enc"], f32)
    xm = np.asarray(inputs["x_mark_enc"], f32)
    mx = np.asarray(inputs["meta_x"], f32)
    tembs = [np.asarray(inputs[f"time_emb{i}"], f32) for i in range(3)]
    membs = [np.asarray(inputs[f"meta_emb{i}"], f32) for i in range(2)]

    tcat = ge[:, :, 4:7].astype(np.int32)
    time_feat = np.concatenate(
        [ge[:, :, :4]] + [tembs[i][tcat[:, :, i]] for i in range(3)], axis=-1
    )  # (B, W, 28)
    mcat = mx[:, 2:4].astype(np.int32)
    meta_feat = np.concatenate(
        [mx[:, :2]] + [membs[i][mcat[:, i]] for i in range(2)], axis=-1
    )  # (B, 34)

    nm = x_enc.mean(axis=1, keepdims=True)  # (B,1,1)
    xc = x_enc - nm
    ns = np.sqrt(xc.var(axis=1, keepdims=True) + 1e-5)
    xn = (xc / ns).astype(f32)  # (B, SEQ, 1)

    teacher = np.zeros((B, W, 1), f32)
    teacher[:, 0] = xn[:, 0]
    teacher[:, 1:SEQ] = xn[:, : SEQ - 1]
    ones = np.ones((B, W, 1), f32)
    ind = np.zeros((B, W, 1), f32)
    ind[:, SEQ:] = 1.0
    xfeat = np.concatenate(
        [teacher, time_feat, xm,
         np.broadcast_to(meta_feat[:, None, :], (B, W, 34)), ones, ind],
        axis=-1,
    )  # (B, W, 69)

    Wi0 = np.asarray(inputs["W_ih0"], np.float64)  # (512, 67)
    Wh0 = np.asarray(inputs["W_hh0"], np.float64)
    Wi1 = np.asarray(inputs["W_ih1"], np.float64)
    Wh1 = np.asarray(inputs["W_hh1"], np.float64)
    b1 = np.asarray(inputs["b_ih0"], np.float64) + np.asarray(inputs["b_hh0"], np.float64)
    b2 = np.asarray(inputs["b_ih1"], np.float64) + np.asarray(inputs["b_hh1"], np.float64)
    meanW = np.asarray(inputs["mean_W"], np.float64)  # (1, 128)
    mean_b = float(np.asarray(inputs["mean_b"]).reshape(()))

    wfb_full = Wi0[:, 0:1] @ (0.5 * meanW)  # consumes H2 = 2*h2
    bias_fb = Wi0[:, 0] * mean_b  # (512,)

    wi0T = _as_blocksT(_perm_rows(Wi0))  # (67, 512)
    wi0T_aug = np.zeros((KX, G4), np.float64)
    wi0T_aug[:IN] = wi0T
    wi0T_aug[IN] = _as_blocksT(_perm_rows(b1)).reshape(G4)  # ones row: bias1
    wi0T_aug[IN + 1] = _as_blocksT(_perm_rows(bias_fb)).reshape(G4)  # indicator
    wh0T = _as_blocksT(_perm_rows(Wh0) * 0.5)  # *0.5: h state is H = 2h
    wi1T = _as_blocksT(_perm_rows(Wi1) * 0.5)
    wh1T = _as_blocksT(_perm_rows(Wh1) * 0.5)
    wfbT = _as_blocksT(_perm_rows(wfb_full))  # (128, 512)

    b2m = _perm_rows(b2).reshape(4, HID)
    bones = np.zeros((4, G4), f32)
    for g in range(4):
        bones[g, g * HID:(g + 1) * HID] = 1.0

    # per-core transposed inputs: (KX, W*BS), feature on partitions
    xt_cores = []
    for c in range(NCORES):
        xf = xfeat[c * BS:(c + 1) * BS]  # (BS, W, KX)
        xt = np.ascontiguousarray(xf.transpose(2, 1, 0)).reshape(KX, W * BS)
        xt_cores.append(xt.astype(BF16))

    # Pack every weight into one (128, WCOLS) tensor -> single DMA.
    wconst = np.zeros((HID, WCOLS), BF16)
    wconst[:KX, WOFF["wi0"]:WOFF["wi0"] + G4] = wi0T_aug
    wconst[:, WOFF["wh0"]:WOFF["wh0"] + G4] = wh0T
    wconst[:, WOFF["wi1"]:WOFF["wi1"] + G4] = wi1T
    wconst[:, WOFF["wh1"]:WOFF["wh1"] + G4] = wh1T
    wconst[:, WOFF["wfb"]:WOFF["wfb"] + G4] = wfbT
    wconst[:4, WOFF["b2m"]:WOFF["b2m"] + HID] = b2m
    wconst[:4, WOFF["bones"]:WOFF["bones"] + G4] = bones

    return dict(
        xt_cores=xt_cores,
        wconst=wconst,
        weights=dict(
            wi0=wi0T_aug.astype(f32), wh0=wh0T.astype(f32),
            wi1=wi1T.astype(f32), wh1=wh1T.astype(f32),
            wfb=wfbT.astype(f32), b2m=b2m.astype(f32), bones=bones,
        ),
        meanW_h=(0.5 * meanW).astype(f32), mean_b=mean_b,
        norm_std=ns.astype(f32), norm_mean=nm.astype(f32),
    )


def host_post(h2_cores, prep):
    """h2_cores: list of (PRED, HID, BS) arrays of H2=2*h2. -> (B, PRED, 1)."""
    meanW_h = prep["meanW_h"][0]  # (HID,)
    out = np.empty((B, PRED, 1), np.float32)
    for c, h2 in enumerate(h2_cores):
        mn = np.einsum("h,thb->bt", meanW_h, h2.astype(np.float32)) + prep["mean_b"]
        out[c * BS:(c + 1) * BS, :, 0] = mn
    out = out * prep["norm_std"] + prep["norm_mean"]
    return out.astype(np.float32)


_CUSTOM_OPS = {}


def _register_op(name, body_fn, ref_fn):
    """Register a custom DVE op via the documented dve_ops extension point."""
    if name in _CUSTOM_OPS:
        return _CUSTOM_OPS[name]
    import concourse.dve_ops as dve_ops
    from concourse.dve_ops import DveOp, get_dve_sub_opcode
    from concourse.dve_spec import Spec, lower
    from concourse.dve_uop import DveOpSpec

    for existing in dve_ops.OPS:
        if existing.name == name:
            _CUSTOM_OPS[name] = existing
            return existing
    spec = Spec(body=body_fn(), reference=ref_fn)
    op = DveOp(name, spec, subdim=False, uops_sha={})
    dve_ops.OPS.append(op)
    dve_ops.CUSTOM_DVE_SPECS[name] = spec
    dve_ops._SUB_OPCODE_FOR_NAME[name] = (
        dve_ops._CUSTOM_DVE_ROW_BASE + len(dve_ops.OPS) - 1)
    shas = {}
    for ver in ("v3", "v4"):
        s = DveOpSpec(name=name, opcode=get_dve_sub_opcode(name),
                      uops=lower(spec, ver=ver), rd1_en=True)
        shas[ver] = s.sha(ver)
    object.__setattr__(op, "uops_sha", shas)
    _CUSTOM_OPS[name] = op
    return op


def _get_tanh7s():
    """out = Src0*(s0 + t*(s1 + t*(imm2 + t*Src1))), t = Src0^2.
    Src1 carries the x^7 coefficient as a broadcast tile."""
    from concourse.dve_spec import Src0, Src1, C0, C1, C2, sq

    def body():
        t = sq(Src0)
        return Src0 * (C0 + t * (C1 + t * (C2 + t * Src1)))

    def ref(in0, in1, s0, s1, imm2):
        x = np.asarray(in0, np.float32)
        t = x * x
        c3 = np.asarray(in1, np.float32)
        return (x * (s0 + t * (s1 + t * (imm2 + t * c3)))).astype(np.float32)

    return _register_op("TANH7S", body, ref)


def _get_tanh5_mul1():
    """out = P*Src1 + P == P*(Src1+1), P = Src0*(s0 + t*(s1 + t*imm2))."""
    from concourse.dve_spec import Src0, Src1, C0, C1, C2, sq

    def body():
        t = sq(Src0)
        P = Src0 * (C0 + t * (C1 + t * C2))
        return P * Src1 + P

    def ref(in0, in1, s0, s1, imm2):
        x = np.asarray(in0, np.float32)
        t = x * x
        P = x * (s0 + t * (s1 + t * imm2))
        return (P * (np.asarray(in1, np.float32) + 1.0)).astype(np.float32)

    return _register_op("TANH5_MUL1", body, ref)


def build_bass():
    import concourse.bass as bass  # noqa: F401
    import concourse.tile as tile
    from concourse import bacc, mybir

    f32 = mybir.dt.float32
    bf16 = mybir.dt.bfloat16
    AF = mybir.ActivationFunctionType
    ALU = mybir.AluOpType
    OFF = 8  # teacher-phase layer-2 lag (decouples the two recurrence chains)
    N_FILL = 3  # pred-phase PE p-state filler matmuls per half-step
    N_FILL_T = 2  # teacher-phase fillers between wi0 and wh0
    tanh7s = _get_tanh7s()
    tanh5m1 = _get_tanh5_mul1()

    nc = bacc.Bacc("TRN2", target_bir_lowering=False, num_devices=NCORES)
    xt_d = nc.dram_tensor("xt", [KX, W * BS], bf16, kind="ExternalInput")
    wc_d = nc.dram_tensor("wconst", [HID, WCOLS], bf16, kind="ExternalInput")
    h2out_d = nc.dram_tensor("h2out", [PRED, HID, BS], bf16, kind="ExternalOutput")

    with tile.TileContext(nc) as tc:
        with (
            tc.tile_pool(name="const", bufs=1) as const,
            tc.tile_pool(name="xin", bufs=3) as xin,
            tc.tile_pool(name="h1p", bufs=OFF + 3) as h1p,
            tc.tile_pool(name="st", bufs=3) as st,
            tc.tile_pool(name="work", bufs=3) as work,
            tc.tile_pool(name="psA", bufs=3, space="PSUM") as psA,
            tc.tile_pool(name="psB", bufs=2, space="PSUM") as psB,
            tc.tile_pool(name="psW", bufs=2, space="PSUM") as psW,
            tc.tile_pool(name="psF", bufs=1, space="PSUM") as psF,
        ):
            wc = const.tile([HID, WCOLS], bf16, tag="wc", name="wc")
            nc.sync.dma_start(out=wc, in_=wc_d[:, :])
            wt = {
                "wi0": wc[:KX, WOFF["wi0"]:WOFF["wi0"] + G4],
                "wh0": wc[:, WOFF["wh0"]:WOFF["wh0"] + G4],
                "wi1": wc[:, WOFF["wi1"]:WOFF["wi1"] + G4],
                "wh1": wc[:, WOFF["wh1"]:WOFF["wh1"] + G4],
                "wfb": wc[:, WOFF["wfb"]:WOFF["wfb"] + G4],
                "b2m": wc[:4, WOFF["b2m"]:WOFF["b2m"] + HID],
                "bones": wc[:4, WOFF["bones"]:WOFF["bones"] + G4],
            }

            def blk(w, g):
                return w[:, g * HID:(g + 1) * HID]

            # x^7 coefficient broadcast tile for the DVE tanh
            g3t = const.tile([HID, 384], bf16, tag="g3t", name="g3t")
            nc.vector.memset(g3t, G3)

            # t_ext chains: [C | Tg | Tf | Ti | To], bf16, 640 cols
            def new_tx(tag):
                return st.tile([HID, 640], bf16, tag=tag, name=tag)

            tx1 = new_tx("tx1")
            tx2 = new_tx("tx2")
            nc.vector.memset(tx1[:, 0:128], 0.0)  # C1 = 0
            nc.vector.memset(tx2[:, 0:128], 0.0)  # C2 = 0

            def new_zero(pool, tag, dt):
                t = pool.tile([HID, BS], dt, tag=tag, name=tag)
                nc.vector.memset(t, 0.0)
                return t

            h1 = new_zero(h1p, "h1", bf16)
            h2 = new_zero(st, "h2", bf16)
            h1_hist = {-1: h1}

            # Load the sigmoid table set first (it also contains tanh), so
            # the kernel pays exactly one ACT_TABLE_LOAD.
            sig0 = work.tile([HID, BS], bf16, tag="S1", name="sig0")
            nc.scalar.activation(out=sig0, in_=wc[:, 0:BS], func=AF.Sigmoid)

            # dense back-to-back matmuls: trip the PE HAM activity window so
            # the array doesn't start cold.
            warm = psA.tile([HID, 384], f32, tag="gA", name="warm")
            for k in range(8):
                nc.tensor.matmul(warm, lhsT=wc[:, 0:HID], rhs=wc[:, 0:384],
                                 start=(k == 0), stop=(k == 7))

            xt_sb = None

            def xcol_for(t):
                nonlocal xt_sb
                if t % X_CHUNK == 0:
                    nsteps = min(X_CHUNK, W - t)
                    xt_sb = xin.tile([KX, X_CHUNK * BS], bf16, tag="xt",
                                     name="xt_sb")
                    nc.sync.dma_start(out=xt_sb[:, :nsteps * BS],
                                      in_=xt_d[:, t * BS:(t + nsteps) * BS])
                return xt_sb[:, (t % X_CHUNK) * BS:(t % X_CHUNK + 1) * BS]

            # gate-matmul groups for the fast cell: A = (g,f,i) -> psA tile;
            # B = o -> psB.  Emitted source-major: each source's B mm first
            # (ACT's To starts early), and the last source (the recurrent,
            # freshest dep) comes after all independent sources, so the PE
            # never stalls with independent work queued behind it.
            def mm_groups(ws_rhs, gA, gB):
                n = len(ws_rhs)
                for k, (wT, rhs) in enumerate(ws_rhs):
                    nc.tensor.matmul(gB, lhsT=blk(wT, 3), rhs=rhs,
                                     start=(k == 0), stop=(k == n - 1))
                    for g in range(3):
                        nc.tensor.matmul(blk(gA, g), lhsT=blk(wT, g), rhs=rhs,
                                         start=(k == 0 and g == 0),
                                         stop=(k == n - 1 and g == 2))

            def fillers(n, ms=None):
                if n <= 0:
                    return
                import contextlib
                ctx = (tc.tile_wait_until(ms=ms) if ms is not None
                       else contextlib.nullcontext())
                with ctx:
                    f = psF.tile([HID, 512], f32, tag="fl", name="fl")
                    for k in range(n):
                        nc.tensor.matmul(f, lhsT=wc[:, 0:HID],
                                         rhs=wc[:, 0:512],
                                         start=(k == 0), stop=(k == n - 1))

            def mm_bias2(gA, gB):
                nc.tensor.matmul(gB, lhsT=wt["b2m"],
                                 rhs=wt["bones"][:, 384:512], start=True,
                                 stop=False)
                nc.tensor.matmul(gA, lhsT=wt["b2m"],
                                 rhs=wt["bones"][:, 0:384], start=True,
                                 stop=False)

            def cell_fast(gA, gB, tx_cur, tx_next, hpool, tag, after=None):
                """Low-latency cell: gate tanh as DVE poly from PSUM; To on
                ACT concurrently (off the critical path).  `after` is an
                ordering-only (no-semaphore) predecessor for the first DVE
                op, used to pin the engine's static order."""
                nc.scalar.activation(out=tx_cur[:, 512:640], in_=gB,
                                     func=AF.Tanh)
                t7 = nc.vector._custom_dve(tanh7s, out=tx_cur[:, 128:512],
                                           in0=gA, in1=g3t[:, 0:384],
                                           s0=G0, s1=G1, imm2=G2)
                uv = work.tile([HID, 256], bf16, tag=f"uv{tag}",
                               name=f"uv{tag}")
                nc.vector.scalar_tensor_tensor(
                    out=uv, in0=tx_cur[:, 256:512], scalar=1.0,
                    in1=tx_cur[:, 0:256], op0=ALU.add, op1=ALU.mult)
                nc.vector.scalar_tensor_tensor(
                    out=tx_next[:, 0:128], in0=uv[:, 0:128], scalar=0.5,
                    in1=uv[:, 128:256], op0=ALU.mult, op1=ALU.add)
                h_new = hpool.tile([HID, BS], bf16, tag=f"h{tag}",
                                   name=f"h{tag}")
                hin = nc.vector._custom_dve(tanh5m1, out=h_new,
                                            in0=tx_next[:, 0:128],
                                            in1=tx_cur[:, 512:640],
                                            s0=H0, s1=H1, imm2=H2)
                return h_new, hin.ins

            def cell_mid(gW, tx_cur, tx_next, hpool, tag, after=None):
                """Lag-hidden cell: single 512-wide ACT tanh over all four
                gate blocks (keeps DVE free for the L1 chain); uv/c/h on DVE
                as in cell_fast.  `after` pins this cell's DVE ops past the
                end of L1's critical chain."""
                nc.scalar.activation(out=tx_cur[:, 128:640], in_=gW,
                                     func=AF.Tanh)
                uv = work.tile([HID, 256], bf16, tag=f"uv{tag}",
                               name=f"uv{tag}")
                uvin = nc.vector.scalar_tensor_tensor(
                    out=uv, in0=tx_cur[:, 256:512], scalar=1.0,
                    in1=tx_cur[:, 0:256], op0=ALU.add, op1=ALU.mult)
                nc.vector.scalar_tensor_tensor(
                    out=tx_next[:, 0:128], in0=uv[:, 0:128], scalar=0.5,
                    in1=uv[:, 128:256], op0=ALU.mult, op1=ALU.add)
                h_new = hpool.tile([HID, BS], bf16, tag=f"h{tag}",
                                   name=f"h{tag}")
                hin = nc.vector._custom_dve(tanh5m1, out=h_new,
                                            in0=tx_next[:, 0:128],
                                            in1=tx_cur[:, 512:640],
                                            s0=H0, s1=H1, imm2=H2)
                return h_new, hin.ins

            # ---------------- teacher phase: L1 stream + L2 stream (lag OFF)
            S_NS = 2400e-6  # logical ms per teacher step (paces fillers)
            h1in = h2in = None
            for i in range(SEQ + OFF):
                j = i - OFF
                if j < 0:
                    # keep the PE HAM window busy until the L2 stream exists
                    wtile = psA.tile([HID, 384], f32, tag="gA", name="wtile")
                    for k in range(6):
                        nc.tensor.matmul(wtile, lhsT=wc[:, 0:HID],
                                         rhs=wc[:, 0:384], start=(k == 0),
                                         stop=(k == 5))
                g2W = None
                if 0 <= j:
                    # L2: bias + wi1 (old h1) first — independent fill work —
                    # wh1 (fresh h2) appended after L1's mms below.
                    g2W = psW.tile([HID, G4], f32, tag="gW", name="g2W")
                    nc.tensor.matmul(g2W, lhsT=wt["b2m"],
                                     rhs=wt["bones"][:, 0:G4], start=True,
                                     stop=False)
                    for g in range(4):
                        nc.tensor.matmul(blk(g2W, g), lhsT=blk(wt["wi1"], g),
                                         rhs=h1_hist[j], start=False,
                                         stop=False)
                g1A = g1B = None
                if i < SEQ:
                    xcol = xcol_for(i)
                    g1A = psA.tile([HID, 384], f32, tag="gA", name="g1A")
                    g1B = psB.tile([HID, BS], f32, tag="gB", name="g1B")
                    # wi0 (independent), p-state filler, then wh0 (fresh dep)
                    nc.tensor.matmul(g1B, lhsT=blk(wt["wi0"], 3), rhs=xcol,
                                     start=True, stop=False)
                    for g in range(3):
                        nc.tensor.matmul(blk(g1A, g), lhsT=blk(wt["wi0"], g),
                                         rhs=xcol, start=(g == 0), stop=False)
                    fillers(N_FILL_T, ms=i * S_NS)
                    h1p_ = h1_hist[i - 1]
                    nc.tensor.matmul(g1B, lhsT=blk(wt["wh0"], 3), rhs=h1p_,
                                     start=False, stop=True)
                    for g in range(3):
                        nc.tensor.matmul(blk(g1A, g), lhsT=blk(wt["wh0"], g),
                                         rhs=h1p_, start=False, stop=(g == 2))
                if g2W is not None:
                    # wh1 @ h2(j-1): the freshest L2 dep, emitted last so the
                    # PE never stalls on it before reaching L1's mms.
                    for g in range(4):
                        nc.tensor.matmul(blk(g2W, g), lhsT=blk(wt["wh1"], g),
                                         rhs=h2, start=False, stop=(g == 3))
                # No-semaphore ordering edges pin the DVE static order to
                # [tanh7 uv1 c1 h1 | uv2 c2 h2] per step: the L2 tail runs in
                # the DVE idle window while the next L1 tanh waits on its
                # matmuls, instead of sandwiching into L1's serial chain.
                if g1A is not None:
                    with tc.high_priority():
                        tx1n = new_tx("tx1")
                        h1_hist[i], h1in = cell_fast(g1A, g1B, tx1, tx1n,
                                                     h1p, "1")
                        tx1 = tx1n
                    h1_hist.pop(i - OFF - 1, None)
                if g2W is not None:
                    with tc.tile_wait_until(ms=i * S_NS + 1800e-6):
                        tx2n = new_tx("tx2")
                        h2, h2in = cell_mid(g2W, tx2, tx2n, st, "2")
                        tx2 = tx2n

            # ---------------- prediction phase: serial, hoisted issue order
            h1 = h1_hist[SEQ - 1]
            P_BASE = (SEQ + OFF) * S_NS
            P_NS = 3800e-6  # logical ms per pred step
            for t in range(SEQ, W):
                pfloor = P_BASE + (t - SEQ) * P_NS
                xcol = xcol_for(t)
                g1A = psA.tile([HID, 384], f32, tag="gA", name="g1A")
                g1B = psB.tile([HID, BS], f32, tag="gB", name="g1B")
                mm_groups([(wt["wi0"], xcol), (wt["wh0"], h1),
                           (wt["wfb"], h2)], g1A, g1B)
                g2A = psA.tile([HID, 384], f32, tag="gA", name="g2A")
                g2B = psB.tile([HID, BS], f32, tag="gB", name="g2B")
                mm_bias2(g2A, g2B)
                nc.tensor.matmul(g2B, lhsT=blk(wt["wh1"], 3), rhs=h2,
                                 start=False, stop=False)
                for g in range(3):
                    nc.tensor.matmul(blk(g2A, g), lhsT=blk(wt["wh1"], g),
                                     rhs=h2, start=False, stop=False)
                fillers(N_FILL, ms=pfloor + 300e-6)
                tx1n = new_tx("tx1")
                h1, _ = cell_fast(g1A, g1B, tx1, tx1n, h1p, "1")
                tx1 = tx1n
                nc.tensor.matmul(g2B, lhsT=blk(wt["wi1"], 3), rhs=h1,
                                 start=False, stop=True)
                for g in range(3):
                    nc.tensor.matmul(blk(g2A, g), lhsT=blk(wt["wi1"], g),
                                     rhs=h1, start=False, stop=(g == 2))
                fillers(N_FILL, ms=pfloor + 2200e-6)
                tx2n = new_tx("tx2")
                h2, _ = cell_fast(g2A, g2B, tx2, tx2n, st, "2")
                tx2 = tx2n
                nc.sync.dma_start(out=h2out_d[t - SEQ], in_=h2)
    nc.compile()
    return nc


_BASS_CACHE = {}


def _get_bass():
    if "nc" not in _BASS_CACHE:
        _BASS_CACHE["nc"] = build_bass()
    return _BASS_CACHE["nc"]


def run(inputs, trace=False):
    """Returns (output, BassKernelResults)."""
    from concourse.bass_utils import run_bass_kernel_spmd

    prep = host_prep(inputs)
    nc = _get_bass()
    in_maps = [{"xt": prep["xt_cores"][c], "wconst": prep["wconst"]}
               for c in range(NCORES)]
    res = run_bass_kernel_spmd(nc, in_maps, core_ids=list(range(NCORES)),
                               trace=trace)
    h2_cores = [r["h2out"] for r in res.results]
    return host_post(h2_cores, prep), res


def kernel(**inputs) -> np.ndarray:
    out, _ = run(inputs, trace=False)
    return out
